# revision 21
# baseline (speedup 1.0000x reference)
"""Trainium2 Bass kernel for nn_CustomABlock (MDTA transformer block).

Per-core: one batch image [C=256, N=4096] per NeuronCore, data-parallel over
B=8; all params replicated.

Design (v2):
  - fp8e4 + DoubleRow K-folding for qkv / proj / mlp1 / mlp2 / gram.
  - depthwise 3x3 for q,k: 9 flat-shift taps on PE as 3 DoubleRow pairs
    (plane step +128) + 3 singles + edge patches; column wrap fixups on
    GPSIMD(Pool); PSUM drains split ACT/DVE.
  - v dwconv absorbed into attn@v: out = sum_t (A diag(wv_t)) @ u_shift_t,
    wrap columns corrected via small strided matmuls + post-drain STTs.
  - residuals carried in bf16 (x + proj -> x1b), output staged f32.
  - l2norm: ACT square(+accum), sqrt(ssq/64), DVE recip -> scale 8/||.||
    folded into the PE transpose via diag matrices; temperature/64 applied
    inside softmax exp (scale/bias), A scaled by 8*wv, rs = 1/(8*sum).
"""

import numpy as np
import ml_dtypes

BF16 = ml_dtypes.bfloat16
FP8 = ml_dtypes.float8_e4m3fn

C = 256
N = 4096
H = W = 64
NH = 8
HID = 307
TS = 512

# tap shift s = 64*dy + dx; pairs have plane step +128
TAP_PAIRS = [((-1, -1), (1, -1)), ((-1, 0), (1, 0)), ((-1, 1), (1, 1))]
TAP_SINGLES = [(0, -1), (0, 0), (0, 1)]
# order in weight tensors: p0a,p0b,p1a,p1b,p2a,p2b,s0,s1,s2
TAP_ORDER = [t for p in TAP_PAIRS for t in p] + TAP_SINGLES
# wrap-column fixups: (tap, out_rows(y0,y1), src is qk3[rows, col])
#   x=0 wraps for dx=-1 taps, x=63 wraps for dx=+1 taps (negated weights)
COL_FIX = [
    ((0, -1), 0, (1, 64), (0, 63), 63),   # dw[1:64,0]  -= w*qk[0:63,63]
    ((-1, -1), 0, (2, 64), (0, 62), 63),  # dw[2:64,0]  -= w*qk[0:62,63]
    ((1, -1), 0, (0, 63), (0, 63), 63),   # dw[0:63,0]  -= w*qk[0:63,63]
    ((0, 1), 63, (0, 63), (1, 64), 0),    # dw[0:63,63] -= w*qk[1:64,0]
    ((-1, 1), 63, (1, 64), (1, 64), 0),   # dw[1:64,63] -= w*qk[1:64,0]
    ((1, 1), 63, (0, 62), (2, 64), 0),    # dw[0:62,63] -= w*qk[2:64,0]
]
# av wrap corrections: (tap, bad col x, y0, y1, src flat offset, stride 64)
AV_FIX = [
    ((0, -1), 0, 1, 64, 63),      # bad n=64y y 1..63, src u[64(y-1)+63]
    ((-1, -1), 0, 2, 64, 63),     # y 2..63, src u[64(y-2)+63]
    ((1, -1), 0, 0, 63, 63),      # y 0..62, src u[64y+63]
    ((0, 1), 63, 0, 63, 64),      # bad n=64y+63 y 0..62, src u[64(y+1)]
    ((-1, 1), 63, 1, 64, 64),     # y 1..63, src u[64y]
    ((1, 1), 63, 0, 62, 128),     # y 0..61, src u[64(y+2)]
]

_CACHE = {}


def _vrange(dy, dx):
    s = 64 * dy + dx
    lo = max(64 * max(0, -dy), -s)
    hi = min(4096 - 64 * max(0, dy), 4096 - s)
    return lo, hi


def _build_bass():
    import concourse.bass as bass
    from concourse import bacc
    from concourse import mybir
    from concourse.ap import AP
    from concourse.tile import TileContext
    from concourse.masks import make_identity

    dt = mybir.dt
    f32 = dt.float32
    bf16 = dt.bfloat16
    fp8 = dt.float8e4
    AF = mybir.ActivationFunctionType
    OP = mybir.AluOpType
    DR = mybir.MatmulPerfMode.DoubleRow

    nc = bacc.Bacc("TRN2")

    xf8_d = nc.dram_tensor("xf8", [128, 2, N], fp8, kind="ExternalInput")
    xb_d = nc.dram_tensor("xb", [128, 2, N], bf16, kind="ExternalInput")
    wqkv_d = nc.dram_tensor("wqkvT", [128, 2, 3 * C], fp8, kind="ExternalInput")
    wtap_d = nc.dram_tensor("wtapd", [128, 4, 9, 128], fp8, kind="ExternalInput")
    wcol_d = nc.dram_tensor("wcol", [128, 4, 6], f32, kind="ExternalInput")
    wv_d = nc.dram_tensor("wv8", [128, 2, 15], f32, kind="ExternalInput")
    wproj_d = nc.dram_tensor("wprojT", [128, 2, C], fp8, kind="ExternalInput")
    wm1_d = nc.dram_tensor("wm1T", [128, 2, 320], fp8, kind="ExternalInput")
    wm2_d = nc.dram_tensor("wm2T", [128, 3, C], fp8, kind="ExternalInput")
    b1_d = nc.dram_tensor("b1", [128, 3], f32, kind="ExternalInput")
    b2_d = nc.dram_tensor("b2", [128, 2], f32, kind="ExternalInput")
    tp_d = nc.dram_tensor("tpos", [128, 2], f32, kind="ExternalInput")
    tn_d = nc.dram_tensor("tneg", [128, 2], f32, kind="ExternalInput")
    out_d = nc.dram_tensor("out", [128, 2, N], f32, kind="ExternalOutput")

    def plane_ap(ap2d, step, count=2):
        """[p, n] 2D AP -> [p, count, n] with an inserted plane dim."""
        dims = list(ap2d.ap)
        return AP(ap2d.tensor, ap2d.offset,
                  [dims[0], [step, count], dims[1]])

    with TileContext(nc) as tc:
        with (
            tc.tile_pool(name="wpool", bufs=1) as wpool,
            tc.tile_pool(name="xpool", bufs=1) as xpool,
            tc.tile_pool(name="qkvp", bufs=3) as qkvp,
            tc.tile_pool(name="dwp", bufs=4) as dwp,
            tc.tile_pool(name="big", bufs=1) as bigp,
            tc.tile_pool(name="qt", bufs=1) as qt_p,
            tc.tile_pool(name="small", bufs=12) as small_p,
            tc.tile_pool(name="apool", bufs=2) as a_p,
            tc.tile_pool(name="avpk", bufs=12) as pk_p,
            tc.tile_pool(name="pbig", bufs=2, space="PSUM") as pbig,
            tc.tile_pool(name="pdw", bufs=2, space="PSUM") as pdw,
            tc.tile_pool(name="psml", bufs=2, space="PSUM") as psml,
        ):
            # ---- input / weight DMAs ----
            xf8_s = xpool.tile([128, 2, N], fp8)
            for kb in range(2):
                nc.sync.dma_start(out=xf8_s[:, kb, :], in_=xf8_d[:, kb, :])
            wqkv_s = wpool.tile([128, 2, 3 * C], fp8)
            nc.sync.dma_start(out=wqkv_s, in_=wqkv_d[:, :, :])
            wtap_s = wpool.tile([128, 4, 9, 128], fp8)
            nc.sync.dma_start(out=wtap_s, in_=wtap_d[:, :, :, :])
            wcol_s = wpool.tile([128, 4, 6], f32)
            nc.sync.dma_start(out=wcol_s, in_=wcol_d[:, :, :])
            wv_s = wpool.tile([128, 2, 15], f32)
            nc.sync.dma_start(out=wv_s, in_=wv_d[:, :, :])
            wproj_s = wpool.tile([128, 2, C], fp8)
            nc.sync.dma_start(out=wproj_s, in_=wproj_d[:, :, :])
            wm1_s = wpool.tile([128, 2, 320], fp8)
            nc.sync.dma_start(out=wm1_s, in_=wm1_d[:, :, :])
            wm2_s = wpool.tile([128, 3, C], fp8)
            nc.sync.dma_start(out=wm2_s, in_=wm2_d[:, :, :])
            b1_s = wpool.tile([128, 3], f32)
            nc.sync.dma_start(out=b1_s, in_=b1_d[:, :])
            b2_s = wpool.tile([128, 2], f32)
            nc.sync.dma_start(out=b2_s, in_=b2_d[:, :])
            tp_s = wpool.tile([128, 2], f32)
            nc.sync.dma_start(out=tp_s, in_=tp_d[:, :])
            tn_s = wpool.tile([128, 2], f32)
            nc.sync.dma_start(out=tn_s, in_=tn_d[:, :])
            xb_s = xpool.tile([128, 2, N], bf16)
            for kb in range(2):
                nc.sync.dma_start(out=xb_s[:, kb, :], in_=xb_d[:, kb, :])

            ident = wpool.tile([128, 128], bf16)
            make_identity(nc, ident)

            qT_s = qt_p.tile([128, 32, C], bf16, tag="qT")
            kT_s = qt_p.tile([128, 32, C], bf16, tag="kT")
            dw_tiles = [None] * 4
            qkv_tiles = [None] * 6
            rs_v = [None, None]
            At_v = [None, None]

            drain_rr = [0]

            def drain(out, in_):
                # distribute PSUM drains between ACT and DVE
                if drain_rr[0] % 2 == 1:
                    nc.vector.tensor_copy(out=out, in_=in_)
                else:
                    nc.scalar.copy(out=out, in_=in_)
                drain_rr[0] += 1

            def do_qkv(ob):
                """qkv channel block ob -> fp8 SBUF tile."""
                q_t = qkvp.tile([128, N], fp8, tag="qkv", name=f"qkv{ob}")
                qkv_tiles[ob] = q_t
                wz = wqkv_s[:, :, ob * 128:(ob + 1) * 128]
                for t in range(4):
                    ps = pbig.tile([128, 1024], f32, tag="pbig", name="ps")
                    for h in range(2):
                        c0 = t * 1024 + h * TS
                        nc.tensor.matmul(
                            ps[:, h * TS:(h + 1) * TS],
                            lhsT=wz,
                            rhs=xf8_s[:, :, c0:c0 + TS],
                            start=True, stop=True, perf_mode=DR)
                    drain(q_t[:, t * 1024:(t + 1) * 1024], ps)

            def emit_taps(out_region, u_t, lhsT_pair, lhsT_single, c0, ts,
                          extra=()):
                """Emit dw tap matmuls for out cols [c0, c0+ts) of flat range.

                out_region: PSUM AP [128, ts]; u_t: fp8 [128, N] source;
                lhsT_pair(j) -> [128,2,128] AP; lhsT_single(i) -> [128,128].
                """
                ops = []
                # (0,0) first: full coverage -> start=True clears everything
                lo, hi = max(c0, 0), min(c0 + ts, N)
                ops.append(("s", 1, lo, hi))
                for j, pair in enumerate(TAP_PAIRS):
                    l0, h0 = _vrange(*pair[0])
                    l1, h1 = _vrange(*pair[1])
                    lo, hi = max(c0, l0, l1), min(c0 + ts, h0, h1)
                    if lo < hi:
                        ops.append(("p", j, lo, hi))
                    # patches: parts of each tap's range the pair missed
                    for ti, tap in enumerate(pair):
                        lv, hv = _vrange(*tap)
                        for plo, phi in ((lv, max(l0, l1)),
                                         (min(h0, h1), hv)):
                            plo2, phi2 = max(c0, plo, lv), min(c0 + ts, phi, hv)
                            if plo2 < phi2:
                                ops.append(("t", (j, ti), plo2, phi2))
                for i, tap in enumerate(TAP_SINGLES):
                    if i == 1:
                        continue
                    lv, hv = _vrange(*tap)
                    lo, hi = max(c0, lv), min(c0 + ts, hv)
                    if lo < hi:
                        ops.append(("s", i, lo, hi))
                ops = list(ops) + list(extra)
                for j, (kind, idx, lo, hi) in enumerate(ops):
                    st = (j == 0)
                    sp = (j == len(ops) - 1)
                    if kind == "p":
                        s0 = 64 * TAP_PAIRS[idx][0][0] + TAP_PAIRS[idx][0][1]
                        nc.tensor.matmul(
                            out_region[:, lo - c0:hi - c0],
                            lhsT=lhsT_pair(idx),
                            rhs=plane_ap(u_t[:, lo + s0:hi + s0], 128),
                            start=st, stop=sp, perf_mode=DR,
                            skip_group_check=True)
                    elif kind == "t":
                        pj, ti = idx
                        dy, dx = TAP_PAIRS[pj][ti]
                        s = 64 * dy + dx
                        nc.tensor.matmul(
                            out_region[:, lo - c0:hi - c0],
                            lhsT=lhsT_single(2 * pj + ti, True),
                            rhs=u_t[:, lo + s:hi + s],
                            start=st, stop=sp, skip_group_check=True)
                    else:
                        dy, dx = TAP_SINGLES[idx]
                        s = 64 * dy + dx
                        nc.tensor.matmul(
                            out_region[:, lo - c0:hi - c0],
                            lhsT=lhsT_single(6 + idx, False),
                            rhs=u_t[:, lo + s:hi + s],
                            start=st, stop=sp, skip_group_check=True)

            def do_dw(b):
                """depthwise conv for q/k block b (0,1=q; 2,3=k)."""
                u_t = qkv_tiles[b]
                dw_t = dwp.tile([128, N], bf16, tag="dw", name=f"dw{b}")
                dw_tiles[b] = dw_t
                dw3 = dw_t.rearrange("p (y x) -> p y x", y=H)
                qk3 = u_t.rearrange("p (y x) -> p y x", y=H)

                def lpair(j):
                    return wtap_s[:, b, 2 * j:2 * j + 2, :]

                def lsingle(i, is_patch):
                    return wtap_s[:, b, i, :]

                for t8 in range(8):
                    pd = pdw.tile([128, TS], f32, tag="pdw", name="pd")
                    emit_taps(pd, u_t, lpair, lsingle, t8 * TS, TS)
                    drain(dw_t[:, t8 * TS:(t8 + 1) * TS], pd)
                # wrap-column fixups (in place, negated weights)
                for fi, (tap, xc, (y0, y1), (sy0, sy1), sx) in enumerate(COL_FIX):
                    nc.vector.scalar_tensor_tensor(
                        out=dw3[:, y0:y1, xc:xc + 1],
                        in0=qk3[:, sy0:sy1, sx:sx + 1],
                        scalar=wcol_s[:, b, fi:fi + 1],
                        in1=dw3[:, y0:y1, xc:xc + 1],
                        op0=OP.mult, op1=OP.add)
                # l2 norm -> diag(8/||row||) fp8 for the scaled transpose
                sq = bigp.tile([128, N], bf16, tag="sq")
                ssq = small_p.tile([128, 1], f32, tag="ssq")
                nc.vector.scalar_tensor_tensor(
                    out=sq, in0=dw_t, scalar=1.0, in1=dw_t,
                    op0=OP.mult, op1=OP.mult, accum_out=ssq)
                nrm = small_p.tile([128, 1], f32, tag="nrm")
                nc.scalar.activation(out=nrm, in_=ssq, func=AF.Sqrt,
                                     scale=1.0 / 64.0)
                rn = small_p.tile([128, 1], f32, tag="rn")
                nc.vector.reciprocal(rn, nrm)
                dg = small_p.tile([128, 128], bf16, tag="dg", name=f"dg{b}")
                nc.vector.tensor_scalar_mul(dg, ident, rn)
                # transpose 32 n-blocks into qT/kT with the scale folded in
                dst = qT_s if b < 2 else kT_s
                cof = (b % 2) * 128
                for g in range(8):
                    tp_t = psml.tile([128, 512], bf16, tag="tp")
                    for i in range(4):
                        nb = g * 4 + i
                        nc.tensor.transpose(
                            tp_t[:, i * 128:(i + 1) * 128],
                            dw_t[:, nb * 128:(nb + 1) * 128], dg)
                    drain(dst[:, g * 4:g * 4 + 4, cof:cof + 128],
                          tp_t.rearrange("p (a b) -> p a b", a=4))

            def do_gram(g):
                pg = psml.tile([128, 512], f32, tag="tp", name="pg")
                co = g * 128
                for nb in range(32):
                    nc.tensor.matmul(
                        pg[:, 0:128],
                        lhsT=qT_s[:, nb, co:co + 128],
                        rhs=kT_s[:, nb, co:co + 128],
                        start=(nb == 0), stop=(nb == 31))
                A_t = a_p.tile([128, 128], bf16, tag="A")
                nc.vector.memset(A_t, 0.0)
                mx = small_p.tile([128, 1], f32, tag="mx")
                sm = small_p.tile([128, 1], f32, tag="sm")
                for h in range(4):
                    r0, r1 = h * 32, h * 32 + 32
                    nc.vector.tensor_reduce(
                        out=mx[r0:r1, :], in_=pg[r0:r1, r0:r1],
                        axis=mybir.AxisListType.X, op=OP.max)
                mxs = small_p.tile([128, 1], f32, tag="mxs")
                nc.vector.tensor_mul(mxs, mx, tn_s[:, g:g + 1])
                for h in range(4):
                    r0, r1 = h * 32, h * 32 + 32
                    nc.scalar.activation(
                        out=A_t[r0:r1, r0:r1], in_=pg[r0:r1, r0:r1],
                        func=AF.Exp, bias=mxs[r0:r1, :],
                        scale=tp_s[r0:r1, g:g + 1],
                        accum_out=sm[r0:r1, :])
                sm8 = small_p.tile([128, 1], f32, tag="sm8")
                nc.vector.tensor_scalar_mul(sm8, sm, 8.0)
                rs = small_p.tile([128, 1], f32, tag="rs")
                nc.vector.reciprocal(rs, sm8)
                rs_v[g] = rs
                pa = psml.tile([128, 512], bf16, tag="tp", name="pa")
                nc.tensor.transpose(pa[:, 0:128], A_t, ident)
                At = a_p.tile([128, 128], bf16, tag="At")
                nc.scalar.copy(out=At, in_=pa[:, 0:128])
                At_v[g] = At

            attn_s = bigp.tile([128, 2, N], fp8, tag="attn")

            def do_av(g):
                u_t = qkv_tiles[4 + g]
                At = At_v[g]
                # scaled A-packs: pairs [128,2,128], singles, negated fixers
                packs = []
                for j in range(3):
                    pk = pk_p.tile([128, 2, 128], fp8, tag=f"pk{g}",
                                   name=f"avp{g}_{j}")
                    for ti in range(2):
                        nc.vector.tensor_scalar_mul(
                            pk[:, ti, :], At, wv_s[:, g, 2 * j + ti:2 * j + ti + 1])
                    packs.append(pk)
                sing = []
                for i in range(3):
                    sg = pk_p.tile([128, 128], fp8, tag=f"pk{g}",
                                   name=f"avs{g}_{i}")
                    nc.vector.tensor_scalar_mul(
                        sg, At, wv_s[:, g, 6 + i:6 + i + 1])
                    sing.append(sg)
                negs = []
                for i in range(6):
                    ng = pk_p.tile([128, 128], fp8, tag=f"pk{g}",
                                   name=f"avn{g}_{i}")
                    nc.vector.tensor_scalar_mul(
                        ng, At, wv_s[:, g, 9 + i:9 + i + 1])
                    negs.append(ng)

                def lpair(j):
                    return packs[j][:, :, :]

                def lsingle(i, is_patch):
                    if i < 6:
                        j, ti = divmod(i, 2)
                        return packs[j][:, ti, :]
                    return sing[i - 6]

                # wrap corrections into a side PSUM tile [128, 6*64]
                pc = psml.tile([128, 512], f32, tag="tp", name=f"pc{g}")
                for k, (tap, xc, y0, y1, soff) in enumerate(AV_FIX):
                    cnt = y1 - y0
                    base = u_t[:, soff:soff + 1]
                    src = AP(base.tensor, base.offset,
                             [list(base.ap[0]), [64, cnt]])
                    nc.tensor.matmul(
                        pc[:, k * 64:k * 64 + cnt], lhsT=negs[k],
                        rhs=src, start=True, stop=True, skip_group_check=True)
                for t in range(4):
                    pv = pbig.tile([128, 1024], f32, tag="pbig", name="pv")
                    for h in range(2):
                        emit_taps(pv[:, h * TS:(h + 1) * TS], u_t, lpair,
                                  lsingle, t * 1024 + h * TS, TS)
                    nc.scalar.mul(attn_s[:, g, t * 1024:(t + 1) * 1024],
                                  pv, rs_v[g])
                att3 = attn_s[:, g, :].rearrange("p (y x) -> p y x", y=H)
                for k, (tap, xc, y0, y1, soff) in enumerate(AV_FIX):
                    cnt = y1 - y0
                    nc.vector.scalar_tensor_tensor(
                        out=att3[:, y0:y1, xc:xc + 1],
                        in0=pc[:, k * 64:k * 64 + cnt].rearrange(
                            "p (n o) -> p n o", o=1),
                        scalar=rs_v[g],
                        in1=att3[:, y0:y1, xc:xc + 1],
                        op0=OP.mult, op1=OP.add)

            # ---- schedule ----
            for b in (0, 2, 1, 3):
                do_qkv(b)
                do_dw(b)
            do_gram(0)
            do_gram(1)
            do_qkv(4)
            do_av(0)
            do_qkv(5)
            do_av(1)

            # ---- tail: proj+resid1 / mlp1 / mlp2+resid2+DMA ----
            x1b = bigp.tile([128, 2, N], bf16, tag="x1b")
            x1f = bigp.tile([128, 2, N], fp8, tag="x1f")
            ys = bigp.tile([128, 3, N], fp8, tag="ys")
            stage = bigp.tile([128, 2, N], f32, tag="stage")
            for t in range(4):
                sl = slice(t * 1024, (t + 1) * 1024)
                for ob in range(2):
                    pp = pbig.tile([128, 1024], f32, tag="pbig", name="pp")
                    for h in range(2):
                        c0 = t * 1024 + h * TS
                        nc.tensor.matmul(
                            pp[:, h * TS:(h + 1) * TS],
                            lhsT=wproj_s[:, :, ob * 128:(ob + 1) * 128],
                            rhs=attn_s[:, :, c0:c0 + TS],
                            start=True, stop=True, perf_mode=DR)
                    nc.vector.tensor_tensor(
                        out=x1b[:, ob, sl], in0=xb_s[:, ob, sl], in1=pp,
                        op=OP.add)
                    nc.gpsimd.tensor_copy(out=x1f[:, ob, sl],
                                          in_=x1b[:, ob, sl])
                for mb in range(3):
                    rows = 128 if mb < 2 else HID - 256
                    pm = pbig.tile([128, 1024], f32, tag="pbig", name="pm")
                    for h in range(2):
                        c0 = t * 1024 + h * TS
                        nc.tensor.matmul(
                            pm[:rows, h * TS:(h + 1) * TS],
                            lhsT=wm1_s[:, :, mb * 128:mb * 128 + rows],
                            rhs=x1f[:, :, c0:c0 + TS],
                            start=True, stop=True, perf_mode=DR)
                    nc.scalar.activation(
                        out=ys[:rows, mb, sl], in_=pm[:rows, :],
                        func=AF.Gelu_apprx_tanh, bias=b1_s[:rows, mb:mb + 1])
                for ob in range(2):
                    pm2 = pbig.tile([128, 1024], f32, tag="pbig", name="pm2")
                    for h in range(2):
                        c0 = t * 1024 + h * TS
                        nc.tensor.matmul(
                            pm2[:, h * TS:(h + 1) * TS],
                            lhsT=wm2_s[:, 0:2, ob * 128:(ob + 1) * 128],
                            rhs=ys[:, 0:2, c0:c0 + TS],
                            start=True, stop=False, perf_mode=DR)
                        nc.tensor.matmul(
                            pm2[:, h * TS:(h + 1) * TS],
                            lhsT=wm2_s[:HID - 256, 2, ob * 128:(ob + 1) * 128],
                            rhs=ys[:HID - 256, 2, c0:c0 + TS],
                            start=False, stop=True)
                    nc.vector.scalar_tensor_tensor(
                        out=stage[:, ob, sl], in0=x1b[:, ob, sl],
                        scalar=b2_s[:, ob:ob + 1], in1=pm2,
                        op0=OP.add, op1=OP.add)
                    nc.sync.dma_start(out=out_d[:, ob, sl],
                                      in_=stage[:, ob, sl])

    return nc


def _prep_shared(w_qkv, w_dw, temperature, w_proj, w_mlp1, b_mlp1, w_mlp2,
                 b_mlp2):
    f32 = np.float32
    shared = {}
    shared["wqkvT"] = np.ascontiguousarray(
        w_qkv.T.reshape(2, 128, 3 * C).transpose(1, 0, 2)).astype(FP8)
    # q/k tap diagonals, TAP_ORDER, 4 blocks of 128 channels
    wt = np.zeros((128, 4, 9, 128), FP8)
    for b in range(4):
        for ti, (dy, dx) in enumerate(TAP_ORDER):
            w = w_dw[b * 128:(b + 1) * 128, 0, dy + 1, dx + 1].astype(f32)
            wt[:, b, ti, :] = np.diag(w).astype(FP8)
    shared["wtapd"] = wt
    # negated col-fix weights (f32)
    wc = np.zeros((128, 4, 6), f32)
    for b in range(4):
        for fi, (tap, *_rest) in enumerate(COL_FIX):
            dy, dx = tap
            wc[:, b, fi] = -w_dw[b * 128:(b + 1) * 128, 0, dy + 1, dx + 1]
    shared["wcol"] = wc
    # v tap scale vectors (8*w), order TAP_ORDER then negated wrap taps
    wv = np.zeros((128, 2, 15), f32)
    for g in range(2):
        ch = slice(512 + g * 128, 512 + (g + 1) * 128)
        for ti, (dy, dx) in enumerate(TAP_ORDER):
            wv[:, g, ti] = 8.0 * w_dw[ch, 0, dy + 1, dx + 1]
        for k, (tap, *_rest) in enumerate(AV_FIX):
            dy, dx = tap
            wv[:, g, 9 + k] = -8.0 * w_dw[ch, 0, dy + 1, dx + 1]
    shared["wv8"] = wv
    shared["wprojT"] = np.ascontiguousarray(
        w_proj.T.reshape(2, 128, C).transpose(1, 0, 2)).astype(FP8)
    wm1p = np.zeros((128, 2, 320), f32)
    wm1p[:, :, :HID] = w_mlp1.T.reshape(2, 128, HID).transpose(1, 0, 2)
    shared["wm1T"] = wm1p.astype(FP8)
    w2 = np.zeros((384, C), f32)
    w2[:HID] = w_mlp2.T
    shared["wm2T"] = np.ascontiguousarray(
        w2.reshape(3, 128, C).transpose(1, 0, 2)).astype(FP8)
    b1 = np.zeros((384,), f32)
    b1[:HID] = b_mlp1
    shared["b1"] = np.ascontiguousarray(b1.reshape(3, 128).T)
    shared["b2"] = np.ascontiguousarray(b_mlp2.astype(f32).reshape(2, 128).T)
    t = temperature.reshape(NH).astype(f32)
    tv = np.zeros((128, 2), f32)
    for g in range(2):
        tv[:, g] = np.repeat(t[g * 4:(g + 1) * 4], 32)
    shared["tpos"] = tv / 64.0
    shared["tneg"] = -tv / 64.0
    return shared


def kernel(x, w_qkv, w_dw, temperature, w_proj, w_mlp1, b_mlp1, w_mlp2, b_mlp2,
           _trace=False):
    from concourse.bass_utils import run_bass_kernel_spmd

    if "nc" not in _CACHE:
        nc = _build_bass()
        nc.finalize()
        _CACHE["nc"] = nc
    nc = _CACHE["nc"]

    x = np.asarray(x, np.float32)
    B = x.shape[0]
    shared = _prep_shared(
        np.asarray(w_qkv, np.float32), np.asarray(w_dw, np.float32),
        np.asarray(temperature, np.float32), np.asarray(w_proj, np.float32),
        np.asarray(w_mlp1, np.float32), np.asarray(b_mlp1, np.float32),
        np.asarray(w_mlp2, np.float32), np.asarray(b_mlp2, np.float32))

    in_maps = []
    for i in range(B):
        m = dict(shared)
        xi = np.ascontiguousarray(x[i].reshape(2, 128, N).transpose(1, 0, 2))
        m["xb"] = xi.astype(BF16)
        m["xf8"] = xi.astype(FP8)
        in_maps.append(m)

    res = run_bass_kernel_spmd(nc, in_maps, core_ids=list(range(B)),
                               trace=_trace)
    outs = np.stack([
        r["out"].transpose(1, 0, 2).reshape(C, H, W) for r in res.results
    ])
    if _trace:
        _CACHE["last_exec_ns"] = res.exec_time_ns
        _CACHE["last_profile"] = res.profile_json
    return outs


# revision 25
# speedup vs baseline: 34.8901x; 34.8901x over previous
"""Trainium2 Bass kernel for nn_CustomABlock (MDTA transformer block).

Per-core: one batch image [C=256, N=4096] per NeuronCore, data-parallel over
B=8; all params replicated.

Design (v2):
  - fp8e4 + DoubleRow K-folding for qkv / proj / mlp1 / mlp2 / gram.
  - depthwise 3x3 for q,k: 9 flat-shift taps on PE as 3 DoubleRow pairs
    (plane step +128) + 3 singles + edge patches; column wrap fixups on
    GPSIMD(Pool); PSUM drains split ACT/DVE.
  - v dwconv absorbed into attn@v: out = sum_t (A diag(wv_t)) @ u_shift_t,
    wrap columns corrected via small strided matmuls + post-drain STTs.
  - residuals carried in bf16 (x + proj -> x1b), output staged f32.
  - l2norm: ACT square(+accum), sqrt(ssq/64), DVE recip -> scale 8/||.||
    folded into the PE transpose via diag matrices; temperature/64 applied
    inside softmax exp (scale/bias), A scaled by 8*wv, rs = 1/(8*sum).
"""

import numpy as np
import ml_dtypes

BF16 = ml_dtypes.bfloat16
FP8 = ml_dtypes.float8_e4m3fn

C = 256
N = 4096
H = W = 64
NH = 8
HID = 307
TS = 512

# tap shift s = 64*dy + dx; pairs have plane step +128
TAP_PAIRS = [((-1, -1), (1, -1)), ((-1, 0), (1, 0)), ((-1, 1), (1, 1))]
TAP_SINGLES = [(0, -1), (0, 0), (0, 1)]
# order in weight tensors: p0a,p0b,p1a,p1b,p2a,p2b,s0,s1,s2
TAP_ORDER = [t for p in TAP_PAIRS for t in p] + TAP_SINGLES
# wrap-column fixups: (tap, out_rows(y0,y1), src is qk3[rows, col])
#   x=0 wraps for dx=-1 taps, x=63 wraps for dx=+1 taps (negated weights)
COL_FIX = [
    ((0, -1), 0, (1, 64), (0, 63), 63),   # dw[1:64,0]  -= w*qk[0:63,63]
    ((-1, -1), 0, (2, 64), (0, 62), 63),  # dw[2:64,0]  -= w*qk[0:62,63]
    ((1, -1), 0, (0, 63), (0, 63), 63),   # dw[0:63,0]  -= w*qk[0:63,63]
    ((0, 1), 63, (0, 63), (1, 64), 0),    # dw[0:63,63] -= w*qk[1:64,0]
    ((-1, 1), 63, (1, 64), (1, 64), 0),   # dw[1:64,63] -= w*qk[1:64,0]
    ((1, 1), 63, (0, 62), (2, 64), 0),    # dw[0:62,63] -= w*qk[2:64,0]
]
# av wrap corrections: (tap, bad col x, y0, y1, src flat offset, stride 64)
AV_FIX = [
    ((0, -1), 0, 1, 64, 63),      # bad n=64y y 1..63, src u[64(y-1)+63]
    ((-1, -1), 0, 2, 64, 63),     # y 2..63, src u[64(y-2)+63]
    ((1, -1), 0, 0, 63, 63),      # y 0..62, src u[64y+63]
    ((0, 1), 63, 0, 63, 64),      # bad n=64y+63 y 0..62, src u[64(y+1)]
    ((-1, 1), 63, 1, 64, 64),     # y 1..63, src u[64y]
    ((1, 1), 63, 0, 62, 128),     # y 0..61, src u[64(y+2)]
]

_CACHE = {}


def _vrange(dy, dx):
    s = 64 * dy + dx
    lo = max(64 * max(0, -dy), -s)
    hi = min(4096 - 64 * max(0, dy), 4096 - s)
    return lo, hi


def _build_bass():
    import concourse.bass as bass
    from concourse import bacc
    from concourse import mybir
    from concourse.ap import AP
    from concourse.tile import TileContext
    from concourse.masks import make_identity

    dt = mybir.dt
    f32 = dt.float32
    bf16 = dt.bfloat16
    fp8 = dt.float8e4
    AF = mybir.ActivationFunctionType
    OP = mybir.AluOpType
    DR = mybir.MatmulPerfMode.DoubleRow

    nc = bacc.Bacc("TRN2")

    xf8_d = nc.dram_tensor("xf8", [128, 2, N], fp8, kind="ExternalInput")
    xb_d = nc.dram_tensor("xb", [128, 2, N], bf16, kind="ExternalInput")
    wqkv_d = nc.dram_tensor("wqkvT", [128, 2, 3 * C], fp8, kind="ExternalInput")
    wtap_d = nc.dram_tensor("wtapd", [128, 4, 9, 128], fp8, kind="ExternalInput")
    wcol_d = nc.dram_tensor("wcol", [128, 4, 6], f32, kind="ExternalInput")
    wv_d = nc.dram_tensor("wv8", [128, 2, 15], f32, kind="ExternalInput")
    wproj_d = nc.dram_tensor("wprojT", [128, 2, C], fp8, kind="ExternalInput")
    wm1_d = nc.dram_tensor("wm1T", [128, 2, 320], fp8, kind="ExternalInput")
    wm2_d = nc.dram_tensor("wm2T", [128, 3, C], fp8, kind="ExternalInput")
    b1_d = nc.dram_tensor("b1", [128, 3], f32, kind="ExternalInput")
    b2_d = nc.dram_tensor("b2", [128, 2], f32, kind="ExternalInput")
    tp_d = nc.dram_tensor("tpos", [128, 2], f32, kind="ExternalInput")
    tn_d = nc.dram_tensor("tneg", [128, 2], f32, kind="ExternalInput")
    out_d = nc.dram_tensor("out", [128, 2, N], f32, kind="ExternalOutput")

    def plane_ap(ap2d, step, count=2):
        """[p, n] 2D AP -> [p, count, n] with an inserted plane dim."""
        dims = list(ap2d.ap)
        return AP(ap2d.tensor, ap2d.offset,
                  [dims[0], [step, count], dims[1]])

    with TileContext(nc) as tc:
        with (
            tc.tile_pool(name="wpool", bufs=1) as wpool,
            tc.tile_pool(name="xpool", bufs=1) as xpool,
            tc.tile_pool(name="qkvp", bufs=3) as qkvp,
            tc.tile_pool(name="dwp", bufs=4) as dwp,
            tc.tile_pool(name="big", bufs=1) as bigp,
            tc.tile_pool(name="qt", bufs=1) as qt_p,
            tc.tile_pool(name="small", bufs=12) as small_p,
            tc.tile_pool(name="apool", bufs=2) as a_p,
            tc.tile_pool(name="avpk", bufs=12) as pk_p,
            tc.tile_pool(name="pbig", bufs=2, space="PSUM") as pbig,
            tc.tile_pool(name="pdw", bufs=2, space="PSUM") as pdw,
            tc.tile_pool(name="psml", bufs=2, space="PSUM") as psml,
        ):
            # ---- input / weight DMAs ----
            xf8_s = xpool.tile([128, 2, N], fp8)
            for kb in range(2):
                nc.sync.dma_start(out=xf8_s[:, kb, :], in_=xf8_d[:, kb, :])
            wqkv_s = wpool.tile([128, 2, 3 * C], fp8)
            nc.sync.dma_start(out=wqkv_s, in_=wqkv_d[:, :, :])
            wtap_s = wpool.tile([128, 4, 9, 128], fp8)
            nc.sync.dma_start(out=wtap_s, in_=wtap_d[:, :, :, :])
            wcol_s = wpool.tile([128, 4, 6], f32)
            nc.sync.dma_start(out=wcol_s, in_=wcol_d[:, :, :])
            wv_s = wpool.tile([128, 2, 15], f32)
            nc.sync.dma_start(out=wv_s, in_=wv_d[:, :, :])
            wproj_s = wpool.tile([128, 2, C], fp8)
            nc.sync.dma_start(out=wproj_s, in_=wproj_d[:, :, :])
            wm1_s = wpool.tile([128, 2, 320], fp8)
            nc.sync.dma_start(out=wm1_s, in_=wm1_d[:, :, :])
            wm2_s = wpool.tile([128, 3, C], fp8)
            nc.sync.dma_start(out=wm2_s, in_=wm2_d[:, :, :])
            b1_s = wpool.tile([128, 3], f32)
            nc.sync.dma_start(out=b1_s, in_=b1_d[:, :])
            b2_s = wpool.tile([128, 2], f32)
            nc.sync.dma_start(out=b2_s, in_=b2_d[:, :])
            tp_s = wpool.tile([128, 2], f32)
            nc.sync.dma_start(out=tp_s, in_=tp_d[:, :])
            tn_s = wpool.tile([128, 2], f32)
            nc.sync.dma_start(out=tn_s, in_=tn_d[:, :])
            xb_s = xpool.tile([128, 2, N], bf16)
            for kb in range(2):
                nc.sync.dma_start(out=xb_s[:, kb, :], in_=xb_d[:, kb, :])

            ident = wpool.tile([128, 128], bf16)
            make_identity(nc, ident)

            qT_s = qt_p.tile([128, 32, C], bf16, tag="qT")
            kT_s = qt_p.tile([128, 32, C], bf16, tag="kT")
            dw_tiles = [None] * 4
            qkv_tiles = [None] * 6
            rs_v = [None, None]
            At_v = [None, None]

            drain_rr = [0]

            def drain(out, in_):
                # distribute PSUM drains between ACT and DVE
                if drain_rr[0] % 5 >= 3:
                    nc.vector.tensor_copy(out=out, in_=in_)
                else:
                    nc.scalar.copy(out=out, in_=in_)
                drain_rr[0] += 1

            def do_qkv(ob):
                """qkv channel block ob -> fp8 SBUF tile."""
                q_t = qkvp.tile([128, N], fp8, tag="qkv", name=f"qkv{ob}")
                qkv_tiles[ob] = q_t
                wz = wqkv_s[:, :, ob * 128:(ob + 1) * 128]
                for t in range(4):
                    ps = pbig.tile([128, 1024], f32, tag="pbig", name="ps")
                    for h in range(2):
                        c0 = t * 1024 + h * TS
                        nc.tensor.matmul(
                            ps[:, h * TS:(h + 1) * TS],
                            lhsT=wz,
                            rhs=xf8_s[:, :, c0:c0 + TS],
                            start=True, stop=True, perf_mode=DR)
                    drain(q_t[:, t * 1024:(t + 1) * 1024], ps)

            def emit_taps(out_region, u_t, lhsT_pair, lhsT_single, c0, ts,
                          extra=()):
                """Emit dw tap matmuls for out cols [c0, c0+ts) of flat range.

                out_region: PSUM AP [128, ts]; u_t: fp8 [128, N] source;
                lhsT_pair(j) -> [128,2,128] AP; lhsT_single(i) -> [128,128].
                """
                ops = []
                # (0,0) first: full coverage -> start=True clears everything
                lo, hi = max(c0, 0), min(c0 + ts, N)
                ops.append(("s", 1, lo, hi))
                for j, pair in enumerate(TAP_PAIRS):
                    l0, h0 = _vrange(*pair[0])
                    l1, h1 = _vrange(*pair[1])
                    lo, hi = max(c0, l0, l1), min(c0 + ts, h0, h1)
                    if lo < hi:
                        ops.append(("p", j, lo, hi))
                    # patches: parts of each tap's range the pair missed
                    for ti, tap in enumerate(pair):
                        lv, hv = _vrange(*tap)
                        for plo, phi in ((lv, max(l0, l1)),
                                         (min(h0, h1), hv)):
                            plo2, phi2 = max(c0, plo, lv), min(c0 + ts, phi, hv)
                            if plo2 < phi2:
                                ops.append(("t", (j, ti), plo2, phi2))
                for i, tap in enumerate(TAP_SINGLES):
                    if i == 1:
                        continue
                    lv, hv = _vrange(*tap)
                    lo, hi = max(c0, lv), min(c0 + ts, hv)
                    if lo < hi:
                        ops.append(("s", i, lo, hi))
                ops = list(ops) + list(extra)
                for j, (kind, idx, lo, hi) in enumerate(ops):
                    st = (j == 0)
                    sp = (j == len(ops) - 1)
                    if kind == "p":
                        s0 = 64 * TAP_PAIRS[idx][0][0] + TAP_PAIRS[idx][0][1]
                        nc.tensor.matmul(
                            out_region[:, lo - c0:hi - c0],
                            lhsT=lhsT_pair(idx),
                            rhs=plane_ap(u_t[:, lo + s0:hi + s0], 128),
                            start=st, stop=sp, perf_mode=DR,
                            skip_group_check=True)
                    elif kind == "t":
                        pj, ti = idx
                        dy, dx = TAP_PAIRS[pj][ti]
                        s = 64 * dy + dx
                        nc.tensor.matmul(
                            out_region[:, lo - c0:hi - c0],
                            lhsT=lhsT_single(2 * pj + ti, True),
                            rhs=u_t[:, lo + s:hi + s],
                            start=st, stop=sp, skip_group_check=True)
                    else:
                        dy, dx = TAP_SINGLES[idx]
                        s = 64 * dy + dx
                        nc.tensor.matmul(
                            out_region[:, lo - c0:hi - c0],
                            lhsT=lhsT_single(6 + idx, False),
                            rhs=u_t[:, lo + s:hi + s],
                            start=st, stop=sp, skip_group_check=True)

            def do_dw(b):
                """depthwise conv for q/k block b (0,1=q; 2,3=k)."""
                u_t = qkv_tiles[b]
                dw_t = dwp.tile([128, N], bf16, tag="dw", name=f"dw{b}")
                dw_tiles[b] = dw_t
                dw3 = dw_t.rearrange("p (y x) -> p y x", y=H)
                qk3 = u_t.rearrange("p (y x) -> p y x", y=H)

                def lpair(j):
                    return wtap_s[:, b, 2 * j:2 * j + 2, :]

                def lsingle(i, is_patch):
                    return wtap_s[:, b, i, :]

                for t8 in range(8):
                    pd = pdw.tile([128, TS], f32, tag="pdw", name="pd")
                    emit_taps(pd, u_t, lpair, lsingle, t8 * TS, TS)
                    drain(dw_t[:, t8 * TS:(t8 + 1) * TS], pd)
                # wrap-column fixups (in place, negated weights)
                for fi, (tap, xc, (y0, y1), (sy0, sy1), sx) in enumerate(COL_FIX):
                    nc.vector.scalar_tensor_tensor(
                        out=dw3[:, y0:y1, xc:xc + 1],
                        in0=qk3[:, sy0:sy1, sx:sx + 1],
                        scalar=wcol_s[:, b, fi:fi + 1],
                        in1=dw3[:, y0:y1, xc:xc + 1],
                        op0=OP.mult, op1=OP.add)
                # l2 norm -> diag(8/||row||) fp8 for the scaled transpose
                sq = bigp.tile([128, N], bf16, tag="sq")
                ssq = small_p.tile([128, 1], f32, tag="ssq")
                nc.vector.scalar_tensor_tensor(
                    out=sq, in0=dw_t, scalar=1.0, in1=dw_t,
                    op0=OP.mult, op1=OP.mult, accum_out=ssq)
                nrm = small_p.tile([128, 1], f32, tag="nrm")
                nc.scalar.activation(out=nrm, in_=ssq, func=AF.Sqrt,
                                     scale=1.0 / 64.0)
                rn = small_p.tile([128, 1], f32, tag="rn")
                nc.vector.reciprocal(rn, nrm)
                dg = small_p.tile([128, 128], bf16, tag="dg", name=f"dg{b}")
                nc.vector.tensor_scalar_mul(dg, ident, rn)
                # transpose 32 n-blocks into qT/kT with the scale folded in
                dst = qT_s if b < 2 else kT_s
                cof = (b % 2) * 128
                for g in range(8):
                    tp_t = psml.tile([128, 512], bf16, tag="tp")
                    for i in range(4):
                        nb = g * 4 + i
                        nc.tensor.transpose(
                            tp_t[:, i * 128:(i + 1) * 128],
                            dw_t[:, nb * 128:(nb + 1) * 128], dg)
                    drain(dst[:, g * 4:g * 4 + 4, cof:cof + 128],
                          tp_t.rearrange("p (a b) -> p a b", a=4))

            def do_gram(g):
                pg = psml.tile([128, 512], f32, tag="tp", name="pg")
                co = g * 128
                for nb in range(32):
                    nc.tensor.matmul(
                        pg[:, 0:128],
                        lhsT=qT_s[:, nb, co:co + 128],
                        rhs=kT_s[:, nb, co:co + 128],
                        start=(nb == 0), stop=(nb == 31))
                A_t = a_p.tile([128, 128], bf16, tag="A")
                nc.vector.memset(A_t, 0.0)
                mx = small_p.tile([128, 1], f32, tag="mx")
                sm = small_p.tile([128, 1], f32, tag="sm")
                for h in range(4):
                    r0, r1 = h * 32, h * 32 + 32
                    nc.vector.tensor_reduce(
                        out=mx[r0:r1, :], in_=pg[r0:r1, r0:r1],
                        axis=mybir.AxisListType.X, op=OP.max)
                mxs = small_p.tile([128, 1], f32, tag="mxs")
                nc.vector.tensor_mul(mxs, mx, tn_s[:, g:g + 1])
                for h in range(4):
                    r0, r1 = h * 32, h * 32 + 32
                    nc.scalar.activation(
                        out=A_t[r0:r1, r0:r1], in_=pg[r0:r1, r0:r1],
                        func=AF.Exp, bias=mxs[r0:r1, :],
                        scale=tp_s[r0:r1, g:g + 1],
                        accum_out=sm[r0:r1, :])
                sm8 = small_p.tile([128, 1], f32, tag="sm8")
                nc.vector.tensor_scalar_mul(sm8, sm, 8.0)
                rs = small_p.tile([128, 1], f32, tag="rs")
                nc.vector.reciprocal(rs, sm8)
                rs_v[g] = rs
                pa = psml.tile([128, 512], bf16, tag="tp", name="pa")
                nc.tensor.transpose(pa[:, 0:128], A_t, ident)
                At = a_p.tile([128, 128], bf16, tag="At")
                nc.scalar.copy(out=At, in_=pa[:, 0:128])
                At_v[g] = At

            attn_s = bigp.tile([128, 2, N], fp8, tag="attn")

            def do_av(g):
                u_t = qkv_tiles[4 + g]
                At = At_v[g]
                # scaled A-packs: pairs [128,2,128], singles, negated fixers
                packs = []
                for j in range(3):
                    pk = pk_p.tile([128, 2, 128], fp8, tag=f"pk{g}",
                                   name=f"avp{g}_{j}")
                    for ti in range(2):
                        nc.vector.tensor_scalar_mul(
                            pk[:, ti, :], At, wv_s[:, g, 2 * j + ti:2 * j + ti + 1])
                    packs.append(pk)
                sing = []
                for i in range(3):
                    sg = pk_p.tile([128, 128], fp8, tag=f"pk{g}",
                                   name=f"avs{g}_{i}")
                    nc.vector.tensor_scalar_mul(
                        sg, At, wv_s[:, g, 6 + i:6 + i + 1])
                    sing.append(sg)
                negs = []
                for i in range(6):
                    ng = pk_p.tile([128, 128], fp8, tag=f"pk{g}",
                                   name=f"avn{g}_{i}")
                    nc.vector.tensor_scalar_mul(
                        ng, At, wv_s[:, g, 9 + i:9 + i + 1])
                    negs.append(ng)

                def lpair(j):
                    return packs[j][:, :, :]

                def lsingle(i, is_patch):
                    if i < 6:
                        j, ti = divmod(i, 2)
                        return packs[j][:, ti, :]
                    return sing[i - 6]

                # wrap corrections into a side PSUM tile [128, 6*64]
                pc = psml.tile([128, 512], f32, tag="tp", name=f"pc{g}")
                for k, (tap, xc, y0, y1, soff) in enumerate(AV_FIX):
                    cnt = y1 - y0
                    base = u_t[:, soff:soff + 1]
                    src = AP(base.tensor, base.offset,
                             [list(base.ap[0]), [64, cnt]])
                    nc.tensor.matmul(
                        pc[:, k * 64:k * 64 + cnt], lhsT=negs[k],
                        rhs=src, start=True, stop=True, skip_group_check=True)
                for t in range(4):
                    pv = pbig.tile([128, 1024], f32, tag="pbig", name="pv")
                    for h in range(2):
                        emit_taps(pv[:, h * TS:(h + 1) * TS], u_t, lpair,
                                  lsingle, t * 1024 + h * TS, TS)
                    nc.scalar.mul(attn_s[:, g, t * 1024:(t + 1) * 1024],
                                  pv, rs_v[g])
                att3 = attn_s[:, g, :].rearrange("p (y x) -> p y x", y=H)
                for k, (tap, xc, y0, y1, soff) in enumerate(AV_FIX):
                    cnt = y1 - y0
                    nc.vector.scalar_tensor_tensor(
                        out=att3[:, y0:y1, xc:xc + 1],
                        in0=pc[:, k * 64:k * 64 + cnt].rearrange(
                            "p (n o) -> p n o", o=1),
                        scalar=rs_v[g],
                        in1=att3[:, y0:y1, xc:xc + 1],
                        op0=OP.mult, op1=OP.add)

            # ---- schedule ----
            for b in (0, 2, 1, 3):
                do_qkv(b)
                do_dw(b)
            do_gram(0)
            do_gram(1)
            do_qkv(4)
            do_av(0)
            do_qkv(5)
            do_av(1)

            # ---- tail: proj+resid1 / mlp1 / mlp2+resid2+DMA ----
            x1b = bigp.tile([128, 2, N], bf16, tag="x1b")
            x1f = bigp.tile([128, 2, N], fp8, tag="x1f")
            ys = bigp.tile([128, 3, N], fp8, tag="ys")
            stage = bigp.tile([128, 2, N], f32, tag="stage")
            for t in range(4):
                sl = slice(t * 1024, (t + 1) * 1024)
                for ob in range(2):
                    pp = pbig.tile([128, 1024], f32, tag="pbig", name="pp")
                    for h in range(2):
                        c0 = t * 1024 + h * TS
                        nc.tensor.matmul(
                            pp[:, h * TS:(h + 1) * TS],
                            lhsT=wproj_s[:, :, ob * 128:(ob + 1) * 128],
                            rhs=attn_s[:, :, c0:c0 + TS],
                            start=True, stop=True, perf_mode=DR)
                    nc.vector.tensor_tensor(
                        out=x1b[:, ob, sl], in0=xb_s[:, ob, sl], in1=pp,
                        op=OP.add)
                    nc.gpsimd.tensor_copy(out=x1f[:, ob, sl],
                                          in_=x1b[:, ob, sl])
                for mb in range(3):
                    rows = 128 if mb < 2 else HID - 256
                    pm = pbig.tile([128, 1024], f32, tag="pbig", name="pm")
                    for h in range(2):
                        c0 = t * 1024 + h * TS
                        nc.tensor.matmul(
                            pm[:rows, h * TS:(h + 1) * TS],
                            lhsT=wm1_s[:, :, mb * 128:mb * 128 + rows],
                            rhs=x1f[:, :, c0:c0 + TS],
                            start=True, stop=True, perf_mode=DR)
                    nc.scalar.activation(
                        out=ys[:rows, mb, sl], in_=pm[:rows, :],
                        func=AF.Gelu_apprx_tanh, bias=b1_s[:rows, mb:mb + 1])
                for ob in range(2):
                    pm2 = pbig.tile([128, 1024], f32, tag="pbig", name="pm2")
                    for h in range(2):
                        c0 = t * 1024 + h * TS
                        nc.tensor.matmul(
                            pm2[:, h * TS:(h + 1) * TS],
                            lhsT=wm2_s[:, 0:2, ob * 128:(ob + 1) * 128],
                            rhs=ys[:, 0:2, c0:c0 + TS],
                            start=True, stop=False, perf_mode=DR)
                        nc.tensor.matmul(
                            pm2[:, h * TS:(h + 1) * TS],
                            lhsT=wm2_s[:HID - 256, 2, ob * 128:(ob + 1) * 128],
                            rhs=ys[:HID - 256, 2, c0:c0 + TS],
                            start=False, stop=True)
                    nc.vector.scalar_tensor_tensor(
                        out=stage[:, ob, sl], in0=x1b[:, ob, sl],
                        scalar=b2_s[:, ob:ob + 1], in1=pm2,
                        op0=OP.add, op1=OP.add)
                    nc.sync.dma_start(out=out_d[:, ob, sl],
                                      in_=stage[:, ob, sl])

    return nc


def _prep_shared(w_qkv, w_dw, temperature, w_proj, w_mlp1, b_mlp1, w_mlp2,
                 b_mlp2):
    f32 = np.float32
    shared = {}
    shared["wqkvT"] = np.ascontiguousarray(
        w_qkv.T.reshape(2, 128, 3 * C).transpose(1, 0, 2)).astype(FP8)
    # q/k tap diagonals, TAP_ORDER, 4 blocks of 128 channels
    wt = np.zeros((128, 4, 9, 128), FP8)
    for b in range(4):
        for ti, (dy, dx) in enumerate(TAP_ORDER):
            w = w_dw[b * 128:(b + 1) * 128, 0, dy + 1, dx + 1].astype(f32)
            wt[:, b, ti, :] = np.diag(w).astype(FP8)
    shared["wtapd"] = wt
    # negated col-fix weights (f32)
    wc = np.zeros((128, 4, 6), f32)
    for b in range(4):
        for fi, (tap, *_rest) in enumerate(COL_FIX):
            dy, dx = tap
            wc[:, b, fi] = -w_dw[b * 128:(b + 1) * 128, 0, dy + 1, dx + 1]
    shared["wcol"] = wc
    # v tap scale vectors (8*w), order TAP_ORDER then negated wrap taps
    wv = np.zeros((128, 2, 15), f32)
    for g in range(2):
        ch = slice(512 + g * 128, 512 + (g + 1) * 128)
        for ti, (dy, dx) in enumerate(TAP_ORDER):
            wv[:, g, ti] = 8.0 * w_dw[ch, 0, dy + 1, dx + 1]
        for k, (tap, *_rest) in enumerate(AV_FIX):
            dy, dx = tap
            wv[:, g, 9 + k] = -8.0 * w_dw[ch, 0, dy + 1, dx + 1]
    shared["wv8"] = wv
    shared["wprojT"] = np.ascontiguousarray(
        w_proj.T.reshape(2, 128, C).transpose(1, 0, 2)).astype(FP8)
    wm1p = np.zeros((128, 2, 320), f32)
    wm1p[:, :, :HID] = w_mlp1.T.reshape(2, 128, HID).transpose(1, 0, 2)
    shared["wm1T"] = wm1p.astype(FP8)
    w2 = np.zeros((384, C), f32)
    w2[:HID] = w_mlp2.T
    shared["wm2T"] = np.ascontiguousarray(
        w2.reshape(3, 128, C).transpose(1, 0, 2)).astype(FP8)
    b1 = np.zeros((384,), f32)
    b1[:HID] = b_mlp1
    shared["b1"] = np.ascontiguousarray(b1.reshape(3, 128).T)
    shared["b2"] = np.ascontiguousarray(b_mlp2.astype(f32).reshape(2, 128).T)
    t = temperature.reshape(NH).astype(f32)
    tv = np.zeros((128, 2), f32)
    for g in range(2):
        tv[:, g] = np.repeat(t[g * 4:(g + 1) * 4], 32)
    shared["tpos"] = tv / 64.0
    shared["tneg"] = -tv / 64.0
    return shared


def kernel(x, w_qkv, w_dw, temperature, w_proj, w_mlp1, b_mlp1, w_mlp2, b_mlp2,
           _trace=False):
    from concourse.bass_utils import run_bass_kernel_spmd

    if "nc" not in _CACHE:
        nc = _build_bass()
        nc.finalize()
        _CACHE["nc"] = nc
    nc = _CACHE["nc"]

    x = np.asarray(x, np.float32)
    B = x.shape[0]
    shared = _prep_shared(
        np.asarray(w_qkv, np.float32), np.asarray(w_dw, np.float32),
        np.asarray(temperature, np.float32), np.asarray(w_proj, np.float32),
        np.asarray(w_mlp1, np.float32), np.asarray(b_mlp1, np.float32),
        np.asarray(w_mlp2, np.float32), np.asarray(b_mlp2, np.float32))

    in_maps = []
    for i in range(B):
        m = dict(shared)
        xi = np.ascontiguousarray(x[i].reshape(2, 128, N).transpose(1, 0, 2))
        m["xb"] = xi.astype(BF16)
        m["xf8"] = xi.astype(FP8)
        in_maps.append(m)

    try:
        res = run_bass_kernel_spmd(nc, in_maps, core_ids=list(range(B)),
                                   trace=_trace)
    except ModuleNotFoundError:
        # axon NTFF profiling hook unavailable in this container
        res = run_bass_kernel_spmd(nc, in_maps, core_ids=list(range(B)),
                                   trace=False)
    outs = np.stack([
        r["out"].transpose(1, 0, 2).reshape(C, H, W) for r in res.results
    ])
    if _trace:
        _CACHE["last_exec_ns"] = res.exec_time_ns
        _CACHE["last_profile"] = res.profile_json
    return outs
